# revision 1
# baseline (speedup 1.0000x reference)
"""Causal multi-head attention for Trainium2, head-sharded across 8 NeuronCores.

Reference computation (note the role swap: softmax rows are indexed by the
*key* position c and run over *query* positions C <= c):
    q = W_Q x ; k = W_K x ; v = W_V x            per head a
    S[c, C] = k[c] . q[C]
    attn = softmax_C( where(C <= c, S, -inf) / sqrt(64) )
    z[c]  = sum_C attn[c, C] v[C]
    out   = sum_a W_O[a] z[a]                     [seq, embed]

Sharding: 16 heads -> 2 heads per core.  Each core computes the partial
output for its two heads; the host sums the 8 partials.

Device-side design (per core, all matmuls in float32r = full-rate fp32):
    xT   [1024, 2048]  x transposed (embed on partitions), streamed in
                       column-chunk-major 256KB pieces
    QT2  [128, 2048]   both heads' q-projection, head-major on partitions
    KT2  [128, 2048]   both heads' k-projection
    V1   [128, 2, 16, 65]  v tiles [C-tile, head, ct, h|ones] (col 64 = 1.0)

    Scores are computed transposed, ST[C, c] = lhsT(QT2).T @ rhs(KT2), so
    the PV contraction (over C) needs no transposes; the ones-column of V1
    yields the softmax denominator as row 64 of the PV accumulator free.

    The whole kernel is one software pipeline over the 4 512-wide c-chunks:
    each chunk's projections run as soon as its xT columns land, its
    attention (both heads' blocks interleaved, diagonal blocks shrunk to
    their unmasked columns) follows immediately, and its output projection
    is deferred into the next chunk's pipeline so no engine drains at the
    chunk boundary.
"""

import numpy as np

import concourse.bacc as bacc
import concourse.mybir as mybir
import concourse.tile as tile
from concourse import bass_utils

BATCH, SEQ, E, NH, H = 1, 2048, 1024, 16, 64
NCORES = 8
HPC = NH // NCORES          # heads per core
H2 = HPC * H                # 128, both heads' h packed
CS = 512                    # c-chunk (free-dim) width
NCS = SEQ // CS             # 4
NKT = E // 128              # 8 k-tiles over embed
NCT = SEQ // 128            # 16 C-tiles over sequence
SCALE = 1.0 / np.sqrt(H)    # 0.125
F32R = mybir.dt.float32r
F32 = mybir.dt.float32

_built = None
CS_SET = None
NBLK_CAP = 99
DEPTH_OVERRIDE = 7


def _build(stage=5):
    nc = bacc.Bacc("TRN2", target_bir_lowering=False, debug=False)

    xT_d = nc.dram_tensor("xT", [E, SEQ], F32R, kind="ExternalInput").ap()
    wq_d = nc.dram_tensor("wq", [E, H2], F32R, kind="ExternalInput").ap()
    wk_d = nc.dram_tensor("wk", [E, H2], F32R, kind="ExternalInput").ap()
    wv_d = nc.dram_tensor("wv", [E, H2], F32R, kind="ExternalInput").ap()
    wo_d = nc.dram_tensor("wo", [H2, E], F32R, kind="ExternalInput").ap()
    ident_d = nc.dram_tensor("ident", [128, 128], F32, kind="ExternalInput").ap()
    masks_d = nc.dram_tensor("masks", [128, 2, CS], F32R, kind="ExternalInput").ap()
    ones1_d = nc.dram_tensor("ones1", [1, H], F32R, kind="ExternalInput").ap()
    vcol_d = nc.dram_tensor("vcol", [128, HPC, NCT, 1], F32R,
                            kind="ExternalInput").ap()
    out_d = nc.dram_tensor("out", [SEQ, E], F32, kind="ExternalOutput").ap()

    def _body(tc):
        with (
            tc.tile_pool(name="persist", bufs=1) as persist,
            tc.tile_pool(name="work", bufs=3) as work,
            tc.tile_pool(name="zpool", bufs=4) as zpool,
            tc.tile_pool(name="opool", bufs=4) as opool,
            tc.tile_pool(name="ps_proj", bufs=2, space="PSUM") as ps_proj,
            tc.tile_pool(name="ps_s", bufs=2, space="PSUM") as ps_s,
            tc.tile_pool(name="ps_z", bufs=2, space="PSUM") as ps_z,
            tc.tile_pool(name="ps_o", bufs=2, space="PSUM") as ps_o,
        ):
            # ---- resident tensors -------------------------------------
            xT = persist.tile([128, NKT, SEQ], F32R)
            wq = persist.tile([128, NKT, H2], F32R)
            wk = persist.tile([128, NKT, H2], F32R)
            wv = persist.tile([128, NKT, H2], F32R)
            wo = persist.tile([128, E], F32R)
            ident = persist.tile([128, 128], F32)
            ones1 = persist.tile([1, H], F32R)
            QT2 = persist.tile([128, SEQ], F32R)
            KT2 = persist.tile([128, SEQ], F32R)
            VT2 = persist.tile([128, SEQ], F32)
            V1 = persist.tile([128, HPC, NCT, H + 1], F32R)
            mask_sb = persist.tile([128, 2, CS], F32R)

            # small DMAs first so weights/constants never queue behind the
            # 8MB xT load; xT is issued column-chunk-major in 256KB pieces
            # so chunk cc's projections start as soon as its columns land
            nc.sync.dma_start(wq[:], wq_d.rearrange("(k p) m -> p k m", p=128))
            for k in range(NKT):
                nc.sync.dma_start(xT[:, k, 0:CS], xT_d[k * 128:(k + 1) * 128, 0:CS])
            nc.sync.dma_start(wk[:], wk_d.rearrange("(k p) m -> p k m", p=128))
            nc.sync.dma_start(wv[:], wv_d.rearrange("(k p) m -> p k m", p=128))
            nc.sync.dma_start(ident[:], ident_d[:])
            nc.sync.dma_start(mask_sb[:], masks_d[:])
            nc.sync.dma_start(ones1[:], ones1_d[:])
            nc.sync.dma_start(V1[:, :, :, H:H + 1], vcol_d[:])
            nc.sync.dma_start(wo[:], wo_d[:])
            for cc in range(1, NCS):
                for k in range(NKT):
                    nc.sync.dma_start(
                        xT[:, k, cc * CS:(cc + 1) * CS],
                        xT_d[k * 128:(k + 1) * 128, cc * CS:(cc + 1) * CS])

            def emit_outproj_unit(cs, z2, mt):
                # out[c, e] = z2.T @ wo: K=128 sums both heads at once
                # (z2 rows 0:64 head0, 64:128 head1; wo rows match)
                for et in range(E // CS):
                    o_ps = ps_o.tile([128, CS], F32, name="o_ps")
                    nc.tensor.matmul(
                        o_ps[:],
                        z2[:, mt * 128:(mt + 1) * 128],
                        wo[:, et * CS:(et + 1) * CS],
                        start=True, stop=True,
                    )
                    o_sb = opool.tile([128, CS], F32, tag="o", name="o_sb")
                    # DVE for all PSUM->SBUF copies: ACT paces the exp chain
                    nc.vector.tensor_copy(o_sb[:], o_ps[:])
                    nc.gpsimd.dma_start(
                        out_d[cs * CS + mt * 128: cs * CS + (mt + 1) * 128,
                              et * CS:(et + 1) * CS],
                        o_sb[:],
                    )

            def outproj_units(cs, z2):
                return [(lambda mt=mt: emit_outproj_unit(cs, z2, mt))
                        for mt in range(CS // 128)]

            def emit_proj(cc, w_sb, dstT):
                # one chunk-projection: accumulate over the 8 embed k-tiles
                c0, c1 = cc * CS, (cc + 1) * CS
                p_ps = ps_proj.tile([128, CS], F32, tag="proj", name="p_ps")
                for k in range(NKT):
                    nc.tensor.matmul(
                        p_ps[:], w_sb[:, k, :], xT[:, k, c0:c1],
                        start=(k == 0), stop=(k == NKT - 1),
                    )
                if dstT is VT2:
                    nc.vector.tensor_copy(dstT[:, c0:c1], p_ps[:])
                else:
                    nc.scalar.activation(dstT[:, c0:c1], p_ps[:],
                                         mybir.ActivationFunctionType.Copy)

            def emit_vtiles(cc, pair):
                # V tiles [C, h] for 2 of the chunk's C-tiles via PE transpose
                for ctl in (2 * pair, 2 * pair + 1):
                    ct = 4 * cc + ctl
                    for hh in range(HPC):
                        t_ps = ps_proj.tile([128, H], F32, tag="proj",
                                            name="t_ps")
                        nc.tensor.transpose(
                            t_ps[:],
                            VT2[hh * H:(hh + 1) * H, ct * 128:(ct + 1) * 128],
                            ident[hh * H:(hh + 1) * H, hh * H:(hh + 1) * H],
                        )
                        nc.vector.tensor_copy(V1[:, hh, ct, 0:H], t_ps[:])

            def proj_units(cc):
                return [
                    lambda: emit_proj(cc, wq, QT2),
                    lambda: emit_proj(cc, wk, KT2),
                    lambda: emit_proj(cc, wv, VT2),
                    lambda: emit_vtiles(cc, 0),
                    lambda: emit_vtiles(cc, 1),
                ]

            filler = []
            cs_list = list(CS_SET if CS_SET is not None else range(NCS))
            for ci, cc in enumerate(cs_list):
                if ci == 0:
                    for f in proj_units(cc):
                        f()

                # ---- attention for cs=cc: both heads' blocks interleaved,
                # diagonal blocks shrunk to their unmasked columns ----------
                cs = cc
                nblk = min(4 * cs + 4, NBLK_CAP)
                z2 = zpool.tile([128, CS], F32R, tag="z", name="z2")
                z_ps = [ps_z.tile([H + 1, CS], F32, tag="zps",
                                  name=f"z_ps{hh}") for hh in range(HPC)]
                # stagger head1 two C-tiles ahead of head0 so the two
                # normalize chains at the end overlap instead of serializing
                OFF = min(2, nblk)
                blocks = []
                for t in range(nblk + OFF):
                    if t < nblk:
                        blocks.append((t, 1))
                    if t >= OFF:
                        blocks.append((t - OFF, 0))
                exp_tiles = {}
                DEPTH = DEPTH_OVERRIDE

                def do_score(i):
                    ct, hh = blocks[i]
                    h0 = hh * H
                    d = ct - 4 * cs
                    # d=3 widened to N=256: f32r runs 4 cyc/row below N=256,
                    # so computing 128 extra masked columns is 2x cheaper
                    off = 256 if d == 3 else (128 * d if d > 0 else 0)
                    n = CS - off
                    s_ps = ps_s.tile([128, CS], F32, tag="s", name="s_ps")
                    nc.tensor.matmul(
                        s_ps[:, 0:n],
                        QT2[h0:h0 + H, ct * 128:(ct + 1) * 128],
                        KT2[h0:h0 + H, cs * CS + off:(cs + 1) * CS],
                        start=True, stop=True,
                    )
                    e_sb = work.tile([128, CS], F32R, tag="exp",
                                     bufs=8, name="e_sb")
                    nc.scalar.activation(
                        e_sb[:, 0:n], s_ps[:, 0:n],
                        mybir.ActivationFunctionType.Exp, scale=SCALE,
                    )
                    if d >= 0:
                        # causal: keep where i + (128d - off) <= j within the
                        # shrunk block; that offset is 0 except for widened d=3
                        mk = 1 if d == 3 else 0
                        nc.vector.tensor_tensor(
                            e_sb[:, 0:n], e_sb[:, 0:n], mask_sb[:, mk, 0:n],
                            op=mybir.AluOpType.mult,
                        )
                    exp_tiles[i] = (e_sb, off, n)

                def do_normalize(hh):
                    # z[h, c] /= z[64, c], via reciprocal + ones-broadcast
                    recip = work.tile([1, CS], F32R, tag="recip",
                                      name="recip")
                    with nc.allow_low_precision("float32r ~ fp32"):
                        nc.vector.reciprocal(recip[:], z_ps[hh][H:H + 1, :])
                    b_ps = ps_s.tile([H, CS], F32, tag="s", name="b_ps")
                    nc.tensor.matmul(b_ps[:], ones1[:], recip[:],
                                     start=True, stop=True)
                    bc_sb = work.tile([H, CS], F32R, tag="bc", name="bc_sb")
                    nc.scalar.activation(bc_sb[:], b_ps[:],
                                         mybir.ActivationFunctionType.Copy)
                    nc.vector.tensor_tensor(
                        z2[hh * H:(hh + 1) * H, :], z_ps[hh][0:H, :],
                        bc_sb[:], op=mybir.AluOpType.mult,
                    )

                def do_pv(i):
                    ct, hh = blocks[i]
                    e_sb, off, n = exp_tiles.pop(i)
                    nc.tensor.matmul(
                        z_ps[hh][:, off:CS], V1[:, hh, ct, :], e_sb[:, 0:n],
                        start=(ct == 0), stop=(ct == nblk - 1),
                    )
                    if ct == nblk - 1:
                        do_normalize(hh)

                # drain filler (prev chunk's outproj + NEXT chunk's
                # projections) into this chunk's score/PV pipeline so PE
                # has independent work while ACT computes the exps
                if ci + 1 < len(cs_list):
                    filler.extend(proj_units(cs_list[ci + 1]))
                nb = len(blocks)
                last_cs = ci == len(cs_list) - 1
                # on the last chunk, hold filler back for the PV tail where
                # no score work is left to hide the exp latency
                hold = min(len(filler), DEPTH) if last_cs else 0
                for i in range(nb):
                    do_score(i)
                    if len(filler) > hold and (i % 2 == 1
                                               or nb - i <= len(filler) - hold):
                        filler.pop(0)()
                    if i >= DEPTH:
                        do_pv(i - DEPTH)
                for i in range(max(0, nb - DEPTH), nb):
                    do_pv(i)
                    if filler:
                        filler.pop(0)()
                while filler:
                    filler.pop(0)()

                if stage <= 4:
                    dbg = opool.tile([128, CS], F32, tag="o", name="dbg")
                    nc.vector.tensor_copy(dbg[:], z2[:])
                    nc.sync.dma_start(
                        out_d[cs * 128:(cs + 1) * 128, 0:CS], dbg[:])
                elif stage >= 5:
                    filler.extend(outproj_units(cs, z2))
            while filler:
                filler.pop(0)()

    with tile.TileContext(nc) as tc:
        _body(tc)
    nc.finalize()
    return nc


def _prep_inputs(x, W_Q, W_K, W_V, W_O):
    x = np.asarray(x, dtype=np.float32)
    W_Q = np.asarray(W_Q, dtype=np.float32)
    W_K = np.asarray(W_K, dtype=np.float32)
    W_V = np.asarray(W_V, dtype=np.float32)
    W_O = np.asarray(W_O, dtype=np.float32)

    xT = np.ascontiguousarray(x[0].T)                      # [E, SEQ]
    in_maps = []
    for c in range(NCORES):
        a0, a1 = HPC * c, HPC * c + 1
        # [E, 2h]: head0's 64 cols then head1's
        wq = np.ascontiguousarray(
            np.concatenate([W_Q[a0].T, W_Q[a1].T], axis=1))
        wk = np.ascontiguousarray(
            np.concatenate([W_K[a0].T, W_K[a1].T], axis=1))
        wv = np.ascontiguousarray(
            np.concatenate([W_V[a0].T, W_V[a1].T], axis=1))
        # [2h, E]
        wo = np.ascontiguousarray(
            np.concatenate([W_O[a0].T, W_O[a1].T], axis=0))
        in_maps.append({"xT": xT, "wq": wq, "wk": wk, "wv": wv, "wo": wo,
                        "ident": _IDENT, "masks": _MASKS, "ones1": _ONES1,
                        "vcol": _VCOL})
    return in_maps


_IDENT = np.eye(128, dtype=np.float32)
_MASKS = np.stack([
    (np.arange(128)[:, None] <= np.arange(CS)[None, :]),
    (np.arange(128)[:, None] + 128 <= np.arange(CS)[None, :]),
], axis=1).astype(np.float32)
_ONES1 = np.ones((1, H), dtype=np.float32)
_VCOL = np.ones((128, HPC, NCT, 1), dtype=np.float32)


def _run(in_maps, trace=False):
    global _built
    if _built is None:
        _built = _build()
    res = bass_utils.run_bass_kernel_spmd(
        _built, in_maps, core_ids=list(range(NCORES)), trace=trace,
    )
    return res


def kernel(x, W_Q, W_K, W_V, W_O):
    in_maps = _prep_inputs(x, W_Q, W_K, W_V, W_O)
    res = _run(in_maps, trace=False)
    acc = np.zeros((SEQ, E), dtype=np.float64)
    for c in range(NCORES):
        acc += res.results[c]["out"]
    return acc.astype(np.float32)[None, :, :]


def kernel_traced(x, W_Q, W_K, W_V, W_O):
    """Like kernel() but also returns a per-core exec-time estimate in ns.

    Prefers a real NTFF profile when the runtime supports it; otherwise
    falls back to the cost-model device-occupancy timeline (TimelineSim),
    since the axon client in this container has no NTFF hook.
    """
    in_maps = _prep_inputs(x, W_Q, W_K, W_V, W_O)
    exec_ns = None
    try:
        res = _run(in_maps, trace=True)
        exec_ns = res.exec_time_ns
    except Exception:
        res = _run(in_maps, trace=False)
    if exec_ns is None:
        from concourse.timeline_sim import TimelineSim
        exec_ns = int(TimelineSim(_built, trace=False).simulate())
    acc = np.zeros((SEQ, E), dtype=np.float64)
    for c in range(NCORES):
        acc += res.results[c]["out"]
    return acc.astype(np.float32)[None, :, :], exec_ns

